# revision 2
# baseline (speedup 1.0000x reference)
"""GateRow kernel for Trainium2 (8 NeuronCores, SPMD gate-sharded).

Problem: out[b, g] = gates[g, 2*x[b, c0[g]] + x[b, c1[g]]]
  x: [16384, 8192] bool, gates: [8192, 4] bool, choices: [8192, 2] int32.

Strategy (per core, gate-sharded GPC=1024, batch bit-packed):
  host:  pack the batch axis 8 bits/byte: TAB = [packbits(x)^T ; ~packbits(x)^T ;
         ones ; zeros]  (16386 rows x 2048 bytes).  Classify each gate's
         truth table into the universal form  f = (a' & b') ^ c'  where
         a'/b'/c' are TAB rows (possibly complemented wires or constants);
         this covers all 16 two-input boolean functions.
  device (per 256-gate chunk):
    1. dma_gather 768 rows (256 a' + 256 b' + 256 c') -> [128, 6, 2048] u8
    2. DVE bitwise AND:  q = a' & b'      (as uint32 lanes)
    3. DVE bitwise XOR:  f = q ^ c'       (as uint32 lanes)
    4. DMA out packed rows [128, 2, 2048] -> HBM
  host:  unpack bits + transpose to [B, G] bool.

Per-core HBM/DMA traffic: 6 MB gather + 2 MB out (vs 48 MB for the
byte-wise batch-sharded design) -- ~20-30 us/core expected.
"""

import sys

for _p in ("/opt/trn_rl_repo", "/opt/pypackages"):
    if _p not in sys.path:
        sys.path.append(_p)

from contextlib import ExitStack
from itertools import product

import numpy as np

import concourse.bass as bass  # noqa: F401  (registers engines)
import concourse.bacc as bacc
import concourse.tile as tile
import concourse.mybir as mybir
from concourse.bass_utils import run_bass_kernel_spmd

B, N, G, NCORES = 16384, 8192, 8192, 8
GPC = G // NCORES      # 1024 gates per core
BPACK = B // 8         # 2048 packed bytes per table row
NCHUNK = 4             # pipeline chunks per core
CG = GPC // NCHUNK     # 256 gates per chunk
NIDX = 3 * CG          # 768 gathered rows per chunk (a', b', c')
ROWS = 2 * N + 2       # x rows, ~x rows, ones, zeros

# ---------------------------------------------------------------------------
# Gate classification:  f(a,b) = (a' & b') ^ c'
#   selector codes: 0 = a, 1 = ~a, 2 = b, 3 = ~b, 4 = ones, 5 = zeros
# ---------------------------------------------------------------------------


def _classify_gates():
    def val(sel, a, b):
        return [a, 1 - a, b, 1 - b, 1, 0][sel]

    forms = np.zeros((16, 3), dtype=np.int64)
    for tt in range(16):
        found = False
        for sa, sb, sc in product([0, 1, 4, 5], [2, 3, 4, 5], range(6)):
            if all(
                ((val(sa, a, b) & val(sb, a, b)) ^ val(sc, a, b))
                == ((tt >> (2 * a + b)) & 1)
                for a in (0, 1)
                for b in (0, 1)
            ):
                forms[tt] = (sa, sb, sc)
                found = True
                break
        assert found, f"truth table {tt} not representable"
    return forms


_FORMS = _classify_gates()

# ---------------------------------------------------------------------------
# Device program (SPMD; all cores run it on their own 1024-gate shard)
# ---------------------------------------------------------------------------


def build_nc():
    u32 = mybir.dt.uint32
    pcall = NIDX // 16  # int16 idx slots per partition per gather call

    nc = bacc.Bacc(
        "TRN2", target_bir_lowering=False, debug=False, num_devices=NCORES
    )
    tab = nc.dram_tensor("tab", [ROWS, BPACK], mybir.dt.uint8, kind="ExternalInput")
    idxs = nc.dram_tensor(
        "idxs", [128, NCHUNK * pcall], mybir.dt.int16, kind="ExternalInput"
    )
    outd = nc.dram_tensor(
        "out", [128, (GPC // 128) * BPACK], mybir.dt.uint8, kind="ExternalOutput"
    )

    with tile.TileContext(nc) as tc, ExitStack() as ctx:
        pconst = ctx.enter_context(tc.tile_pool(name="const", bufs=1))
        pg = ctx.enter_context(tc.tile_pool(name="gather", bufs=2))
        pq = ctx.enter_context(tc.tile_pool(name="and", bufs=2))
        po = ctx.enter_context(tc.tile_pool(name="out", bufs=2))

        idx_t = pconst.tile([128, idxs.shape[1]], mybir.dt.int16)
        nc.sync.dma_start(idx_t[:], idxs[:])

        for k in range(NCHUNK):
            g_t = pg.tile([128, 6, BPACK], mybir.dt.uint8, tag="g")
            nc.gpsimd.dma_gather(
                g_t[:],
                tab[:],
                idx_t[:, k * pcall : (k + 1) * pcall],
                NIDX,
                NIDX,
                BPACK,
                single_packet=False,
            )
            q_t = pq.tile([128, 2, BPACK], mybir.dt.uint8, tag="q")
            nc.vector.tensor_tensor(
                q_t[:].bitcast(u32),
                g_t[:, 0:2, :].bitcast(u32),
                g_t[:, 2:4, :].bitcast(u32),
                mybir.AluOpType.bitwise_and,
            )
            o_t = po.tile([128, 2, BPACK], mybir.dt.uint8, tag="o")
            nc.vector.tensor_tensor(
                o_t[:].bitcast(u32),
                q_t[:].bitcast(u32),
                g_t[:, 4:6, :].bitcast(u32),
                mybir.AluOpType.bitwise_xor,
            )
            nc.sync.dma_start(
                outd[:, k * 2 * BPACK : (k + 1) * 2 * BPACK], o_t[:]
            )
    nc.compile()
    return nc


# ---------------------------------------------------------------------------
# Host-side input prep / output assembly
# ---------------------------------------------------------------------------


def _prep_inputs(x, gates, choices):
    x8 = np.asarray(x, dtype=np.uint8)
    g8 = np.asarray(gates, dtype=np.uint8)
    ch = np.asarray(choices, dtype=np.int64)

    # table: packed x^T, complemented rows, ones, zeros
    xp = np.packbits(x8, axis=0, bitorder="little")  # [B/8, N] -> bit j of [k, w] = x[8k+j, w]
    tabx = np.ascontiguousarray(xp.T)                # [N, BPACK]
    tab_full = np.empty((ROWS, BPACK), dtype=np.uint8)
    tab_full[:N] = tabx
    tab_full[N : 2 * N] = tabx ^ 0xFF
    tab_full[2 * N] = 0xFF
    tab_full[2 * N + 1] = 0

    # per-gate row selectors
    tt = (g8[:, 0] | (g8[:, 1] << 1) | (g8[:, 2] << 2) | (g8[:, 3] << 3)).astype(
        np.int64
    )
    sel = _FORMS[tt]  # [G, 3]
    c0, c1 = ch[:, 0], ch[:, 1]

    def row_of(code):
        return np.select(
            [code == 0, code == 1, code == 2, code == 3, code == 4, code == 5],
            [c0, N + c0, c1, N + c1,
             np.full(G, 2 * N, np.int64), np.full(G, 2 * N + 1, np.int64)],
        )

    rowA, rowB, rowC = (row_of(sel[:, j]) for j in range(3))

    # dma_gather wrapped index layout: idx i -> partition i%16, slot i//16,
    # replicated across the 8 gpsimd cores (x8 partitions).
    in_maps = []
    for c in range(NCORES):
        cols = []
        for k in range(NCHUNK):
            gl = slice(c * GPC + k * CG, c * GPC + (k + 1) * CG)
            flat = np.concatenate([rowA[gl], rowB[gl], rowC[gl]]).astype(np.int16)
            wrapped = flat.reshape(-1, 16).T  # [16, NIDX/16]
            cols.append(np.tile(wrapped, (8, 1)))  # [128, NIDX/16]
        idxs_np = np.ascontiguousarray(np.concatenate(cols, axis=1))
        in_maps.append({"tab": tab_full, "idxs": idxs_np})
    return in_maps


def _assemble(results):
    parts = []
    for c in range(NCORES):
        o = results[c]["out"]  # [128, 8*BPACK] u8; row p, chunk k, slot j
        parts.append(
            o.reshape(128, NCHUNK, 2, BPACK)
            .transpose(1, 2, 0, 3)
            .reshape(GPC, BPACK)
        )
    packed = np.concatenate(parts, axis=0)  # [G, BPACK], gate-major
    return np.unpackbits(packed, axis=1, bitorder="little").T.astype(bool)


# ---------------------------------------------------------------------------
# Entry point
# ---------------------------------------------------------------------------

_NC_CACHE = {}


def _get_nc():
    if "nc" not in _NC_CACHE:
        _NC_CACHE["nc"] = build_nc()
    return _NC_CACHE["nc"]


def kernel(x, gates, choices):
    in_maps = _prep_inputs(x, gates, choices)
    nc = _get_nc()
    res = run_bass_kernel_spmd(nc, in_maps, list(range(NCORES)))
    return _assemble(res.results)
